# revision 9
# baseline (speedup 1.0000x reference)
"""TRN2 Bass kernel for MultiHeadAttention (relu attention, 8 heads, D=1024).

Sharding: 8 cores = 2 batches x 4 head-groups (2 heads each).
Each core computes, for its (batch b, heads hg*2+{0,1}):
  QT/KT/VT projections (f32r matmuls, fp32 accumulate),
  AT = relu(scale * K Q^T)  (the attention matrix TRANSPOSED, [key, query]),
  ctxT = V^T @ AT-chain     (context transposed [dv, q]),
  out_partial = ctxT^T @ Wo_rows  (partial over this core's 256 dv rows).
Host side: pre-transposes q/k/v activations, pre-arranges weight slices
into SBUF layout, transposes AT back to [q, k] when assembling
attn_weights, and sums the 4 partial outputs per batch (+ bo).

Returns (output, attn_weights) matching the reference module.
"""

import numpy as np

import concourse.bacc as bacc
import concourse.mybir as mybir
from concourse import tile
from concourse.bass_utils import run_bass_kernel_spmd

B = 2
S = 1024 * 2  # 2048 sequence
D = 1024
H = 8
HD = 128  # head dim
NCORES = 8
HPC = 2  # heads per core
DG = HPC * HD  # 256, d-slice per core
SCALE = float(1.0 / np.sqrt(HD))

P = 128
DT = D // P        # 8 contraction tiles over d_model
KT = S // P        # 16 key tiles
QC = S // 512      # 4 query chunks of 512
F32R = mybir.dt.float32r

_CACHED_NC = None


def _build_nc():
    nc = bacc.Bacc("TRN2", target_bir_lowering=False, debug=False)

    # ---- I/O ----  (w* arrive pre-arranged as [P, DT, DG] / [P, HPC, D])
    xqT = nc.dram_tensor("xqT", [D, S], F32R, kind="ExternalInput")
    xkT = nc.dram_tensor("xkT", [D, S], F32R, kind="ExternalInput")
    xvT = nc.dram_tensor("xvT", [D, S], F32R, kind="ExternalInput")
    wq = nc.dram_tensor("wq", [P, DT, DG], F32R, kind="ExternalInput")
    wk = nc.dram_tensor("wk", [P, DT, DG], F32R, kind="ExternalInput")
    wv = nc.dram_tensor("wv", [P, DT, DG], F32R, kind="ExternalInput")
    wo = nc.dram_tensor("wo", [P, HPC, D], F32R, kind="ExternalInput")
    bqs = nc.dram_tensor("bqs", [P, HPC], F32R, kind="ExternalInput")  # bq*scale
    bk2 = nc.dram_tensor("bk2", [P, HPC], F32R, kind="ExternalInput")
    bv2 = nc.dram_tensor("bv2", [P, HPC], F32R, kind="ExternalInput")
    ident = nc.dram_tensor("ident", [P, P], F32R, kind="ExternalInput")

    attn_t = nc.dram_tensor("attn_t", [HPC, S, S], F32R, kind="ExternalOutput")
    out_partial = nc.dram_tensor("out_partial", [S, D], F32R, kind="ExternalOutput")

    Identity = mybir.ActivationFunctionType.Identity
    Copy = mybir.ActivationFunctionType.Copy
    Relu = mybir.ActivationFunctionType.Relu

    with tile.TileContext(nc) as tc, (
        tc.tile_pool(name="const", bufs=1)
    ) as cpool, tc.tile_pool(name="persist", bufs=1) as ppool:
        # ---- constants (ACT hwdge queue; x stream alternates queues) ----
        wk_sb = cpool.tile([P, DT, DG], F32R, tag="wk")
        wv_sb = cpool.tile([P, DT, DG], F32R, tag="wv")
        wq_sb = cpool.tile([P, DT, DG], F32R, tag="wq")
        wo_sb = cpool.tile([P, HPC, D], F32R, tag="wo")
        bqs_sb = cpool.tile([P, HPC], F32R, tag="bqs")
        bk2_sb = cpool.tile([P, HPC], F32R, tag="bk2")
        bv2_sb = cpool.tile([P, HPC], F32R, tag="bv2")
        id_sb = cpool.tile([P, P], F32R, tag="ident")

        nc.scalar.dma_start(out=wk_sb[:], in_=wk.ap())
        nc.scalar.dma_start(out=bk2_sb[:], in_=bk2.ap())
        nc.scalar.dma_start(out=bv2_sb[:], in_=bv2.ap())
        nc.scalar.dma_start(out=bqs_sb[:], in_=bqs.ap())
        nc.scalar.dma_start(out=id_sb[:], in_=ident.ap())
        nc.scalar.dma_start(out=wv_sb[:], in_=wv.ap())
        nc.scalar.dma_start(out=wq_sb[:], in_=wq.ap())
        nc.scalar.dma_start(out=wo_sb[:], in_=wo.ap())

        # ---- persistent activations ----
        qt_all = ppool.tile([P, HPC, S], F32R, tag="qt")   # Q^T per head
        kt_all = ppool.tile([P, HPC, S], F32R, tag="kt")   # K^T per head
        vt_all = ppool.tile([P, HPC, S], F32R, tag="vt")   # V^T per head
        v_all = ppool.tile([P, KT, DG], F32R, tag="v")     # V [k, dv]
        ctxT = ppool.tile([P, HPC, S], F32R, tag="ctxT")   # context^T

        # ---- projections: x^T streamed, 8 PSUM banks per phase ----
        with (
            tc.tile_pool(name="xs", bufs=4) as xpool,
            tc.tile_pool(name="pproj", bufs=8, space="PSUM") as ps_proj,
        ):
            def proj_phase(x_dram, w_sb, out_all, bias_sb, scale, phase):
                accs = [
                    ps_proj.tile([P, 512], mybir.dt.float32, tag="proj",
                                 name=f"acc_{phase}_{i}")
                    for i in range(HPC * QC)
                ]
                for dt_i in range(DT):
                    xt = xpool.tile([P, S], F32R, tag="xs", name=f"x_{phase}_{dt_i}")
                    eng = nc.sync if dt_i % 2 == 0 else nc.gpsimd
                    eng.dma_start(out=xt[:],
                                  in_=x_dram.ap()[dt_i * P:(dt_i + 1) * P, :])
                    for h in range(HPC):
                        for qc in range(QC):
                            nc.tensor.matmul(
                                accs[h * QC + qc][:],
                                w_sb[:, dt_i, h * HD:(h + 1) * HD],
                                xt[:, qc * 512:(qc + 1) * 512],
                                start=(dt_i == 0),
                                stop=(dt_i == DT - 1),
                            )
                for h in range(HPC):
                    for qc in range(QC):
                        nc.scalar.activation(
                            out_all[:, h, qc * 512:(qc + 1) * 512],
                            accs[h * QC + qc][:],
                            Identity,
                            bias=bias_sb[:, h:h + 1],
                            scale=scale,
                        )

            proj_phase(xkT, wk_sb, kt_all, bk2_sb, 1.0, "k")
            proj_phase(xvT, wv_sb, vt_all, bv2_sb, 1.0, "v")

            # transpose V^T -> V [k, dv] via PE (interleaved with Q-proj DMA)
            for h in range(HPC):
                for kt_i in range(KT):
                    tr = ps_proj.tile([P, P], F32R, tag="proj",
                                      name=f"tr_{h}_{kt_i}")
                    nc.tensor.transpose(
                        tr[:], vt_all[:, h, kt_i * P:(kt_i + 1) * P], id_sb[:]
                    )
                    nc.vector.tensor_copy(
                        v_all[:, kt_i, h * HD:(h + 1) * HD], tr[:]
                    )

            proj_phase(xqT, wq_sb, qt_all, bqs_sb, SCALE, "q")

        # ---- attention per head + output projection ----
        with (
            tc.tile_pool(name="attn_ps", bufs=1, space="PSUM") as ps_attn,
            tc.tile_pool(name="attn_sb", bufs=1) as apool,
        ):
            for h in range(HPC):
                ctx_ps = [
                    ps_attn.tile([P, 512], mybir.dt.float32, tag="ctx", bufs=4,
                                 name=f"ctxps_{h}_{qc}")
                    for qc in range(QC)
                ]
                for kt_i in range(KT):
                    for half in range(2):
                        at_ps = ps_attn.tile(
                            [P, 1024], mybir.dt.float32, tag="at", bufs=2,
                            name=f"atps_{h}_{kt_i}_{half}",
                        )
                        for sub in range(2):
                            qc = half * 2 + sub
                            nc.tensor.matmul(
                                at_ps[:, sub * 512:(sub + 1) * 512],
                                kt_all[:, h, kt_i * P:(kt_i + 1) * P],
                                qt_all[:, h, qc * 512:(qc + 1) * 512],
                                start=True,
                                stop=True,
                            )
                        at_sb = apool.tile([P, 1024], F32R, tag="at_sb", bufs=4,
                                           name=f"atsb_{h}_{kt_i}_{half}")
                        if half == 0:
                            nc.scalar.activation(at_sb[:], at_ps[:], Relu)
                        else:
                            nc.vector.tensor_scalar_max(at_sb[:], at_ps[:], 0.0)
                        eng = nc.sync if half == 0 else nc.gpsimd
                        eng.dma_start(
                            out=attn_t.ap()[h, kt_i * P:(kt_i + 1) * P,
                                            half * 1024:(half + 1) * 1024],
                            in_=at_sb[:],
                        )
                        for sub in range(2):
                            qc = half * 2 + sub
                            nc.tensor.matmul(
                                ctx_ps[qc][:],
                                v_all[:, kt_i, h * HD:(h + 1) * HD],
                                at_sb[:, sub * 512:(sub + 1) * 512],
                                start=(kt_i == 0),
                                stop=(kt_i == KT - 1),
                            )
                for qc in range(QC):
                    nc.scalar.activation(
                        ctxT[:, h, qc * 512:(qc + 1) * 512], ctx_ps[qc][:], Copy
                    )

            # output projection
            for st in range(KT):
                o_ps = ps_attn.tile([P, D], mybir.dt.float32, tag="at", bufs=2,
                                    name=f"ops_{st}")
                for oc in range(2):
                    for h in range(HPC):
                        nc.tensor.matmul(
                            o_ps[:, oc * 512:(oc + 1) * 512],
                            ctxT[:, h, st * P:(st + 1) * P],
                            wo_sb[:, h, oc * 512:(oc + 1) * 512],
                            start=(h == 0),
                            stop=(h == HPC - 1),
                        )
                o_sb = apool.tile([P, D], F32R, tag="o_sb", bufs=4,
                                  name=f"osb_{st}")
                nc.vector.tensor_copy(o_sb[:], o_ps[:])
                eng = nc.sync if st % 2 == 0 else nc.gpsimd
                eng.dma_start(
                    out=out_partial.ap()[st * P:(st + 1) * P, :], in_=o_sb[:]
                )

    nc.compile()
    return nc


def _get_nc():
    global _CACHED_NC
    if _CACHED_NC is None:
        _CACHED_NC = _build_nc()
    return _CACHED_NC


def _warr(w):
    # [D, DG] -> [P, DT, DG] matching SBUF weight layout
    return np.ascontiguousarray(w.reshape(DT, P, -1).transpose(1, 0, 2))


def kernel(query, key, value, Wq, bq, Wk, bk, Wv, bv, Wo, bo):
    query = np.ascontiguousarray(np.asarray(query, dtype=np.float32))
    key = np.ascontiguousarray(np.asarray(key, dtype=np.float32))
    value = np.ascontiguousarray(np.asarray(value, dtype=np.float32))
    Wq = np.asarray(Wq, dtype=np.float32)
    bq = np.asarray(bq, dtype=np.float32)
    Wk = np.asarray(Wk, dtype=np.float32)
    bk = np.asarray(bk, dtype=np.float32)
    Wv = np.asarray(Wv, dtype=np.float32)
    bv = np.asarray(bv, dtype=np.float32)
    Wo = np.asarray(Wo, dtype=np.float32)
    bo = np.asarray(bo, dtype=np.float32)

    nc = _get_nc()

    xT = {}
    for name, x in (("q", query), ("k", key), ("v", value)):
        xT[name] = [np.ascontiguousarray(x[b].T) for b in range(B)]
    ident = np.eye(P, dtype=np.float32)

    in_maps = []
    for c in range(NCORES):
        b, hg = divmod(c, 4)
        sl = slice(hg * DG, (hg + 1) * DG)
        in_maps.append({
            "xqT": xT["q"][b],
            "xkT": xT["k"][b],
            "xvT": xT["v"][b],
            "wq": _warr(Wq[:, sl]),
            "wk": _warr(Wk[:, sl]),
            "wv": _warr(Wv[:, sl]),
            "wo": np.ascontiguousarray(
                Wo[sl, :].reshape(HPC, P, D).transpose(1, 0, 2)
            ),
            "bqs": np.ascontiguousarray((bq[sl] * SCALE).reshape(HPC, P).T),
            "bk2": np.ascontiguousarray(bk[sl].reshape(HPC, P).T),
            "bv2": np.ascontiguousarray(bv[sl].reshape(HPC, P).T),
            "ident": ident,
        })

    res = run_bass_kernel_spmd(nc, in_maps, core_ids=list(range(NCORES)))

    attn = np.empty((B, H, S, S), dtype=np.float32)
    output = np.zeros((B, S, D), dtype=np.float32)
    for c in range(NCORES):
        b, hg = divmod(c, 4)
        r = res.results[c]
        at = r["attn_t"]  # [HPC, S(key), S(query)]
        for h in range(HPC):
            attn[b, hg * HPC + h] = at[h].T
        output[b] += r["out_partial"]
    output += bo[None, None, :]
    return output, attn


# revision 10
# speedup vs baseline: 1.1999x; 1.1999x over previous
"""TRN2 Bass kernel for MultiHeadAttention (relu attention, 8 heads, D=1024).

Sharding: 8 cores = 2 batches x 4 head-groups (2 heads each).
Each core computes, for its (batch b, heads hg*2+{0,1}):
  QT/KT/VT projections (f32r matmuls, fp32 accumulate),
  AT = relu(scale * K Q^T)  (the attention matrix TRANSPOSED, [key, query]),
  ctxT = V^T @ AT-chain     (context transposed [dv, q]),
  out_partial = ctxT^T @ Wo_rows  (partial over this core's 256 dv rows).
Host side: pre-transposes q/k/v activations, pre-arranges weight slices
into SBUF layout, transposes AT back to [q, k] when assembling
attn_weights, and sums the 4 partial outputs per batch (+ bo).

Returns (output, attn_weights) matching the reference module.
"""

import numpy as np

import concourse.bacc as bacc
import concourse.mybir as mybir
from concourse import tile
from concourse.bass_utils import run_bass_kernel_spmd

B = 2
S = 1024 * 2  # 2048 sequence
D = 1024
H = 8
HD = 128  # head dim
NCORES = 8
HPC = 2  # heads per core
DG = HPC * HD  # 256, d-slice per core
SCALE = float(1.0 / np.sqrt(HD))

P = 128
DT = D // P        # 8 contraction tiles over d_model
KT = S // P        # 16 key tiles
QC = S // 512      # 4 query chunks of 512
F32R = mybir.dt.float32r

_CACHED_NC = None


def _build_nc():
    nc = bacc.Bacc("TRN2", target_bir_lowering=False, debug=False)

    # ---- I/O ----  (w* arrive pre-arranged as [P, DT, DG] / [P, HPC, D])
    xqT = nc.dram_tensor("xqT", [D, S], F32R, kind="ExternalInput")
    xkT = nc.dram_tensor("xkT", [D, S], F32R, kind="ExternalInput")
    xvT = nc.dram_tensor("xvT", [D, S], F32R, kind="ExternalInput")
    wq = nc.dram_tensor("wq", [P, DT, DG], F32R, kind="ExternalInput")
    wk = nc.dram_tensor("wk", [P, DT, DG], F32R, kind="ExternalInput")
    wv = nc.dram_tensor("wv", [P, DT, DG], F32R, kind="ExternalInput")
    wo = nc.dram_tensor("wo", [P, HPC, D], F32R, kind="ExternalInput")
    bqs = nc.dram_tensor("bqs", [P, HPC], F32R, kind="ExternalInput")  # bq*scale
    bk2 = nc.dram_tensor("bk2", [P, HPC], F32R, kind="ExternalInput")
    bv2 = nc.dram_tensor("bv2", [P, HPC], F32R, kind="ExternalInput")
    ident = nc.dram_tensor("ident", [P, P], F32R, kind="ExternalInput")

    attn_t = nc.dram_tensor("attn_t", [HPC, S, S], F32R, kind="ExternalOutput")
    out_partial = nc.dram_tensor("out_partial", [S, D], F32R, kind="ExternalOutput")

    Identity = mybir.ActivationFunctionType.Identity
    Copy = mybir.ActivationFunctionType.Copy
    Relu = mybir.ActivationFunctionType.Relu

    with tile.TileContext(nc) as tc, (
        tc.tile_pool(name="const", bufs=1)
    ) as cpool, tc.tile_pool(name="persist", bufs=1) as ppool:
        # ---- constants (ACT hwdge queue; x stream alternates queues) ----
        wk_sb = cpool.tile([P, DT, DG], F32R, tag="wk")
        wv_sb = cpool.tile([P, DT, DG], F32R, tag="wv")
        wq_sb = cpool.tile([P, DT, DG], F32R, tag="wq")
        wo_sb = cpool.tile([P, HPC, D], F32R, tag="wo")
        bqs_sb = cpool.tile([P, HPC], F32R, tag="bqs")
        bk2_sb = cpool.tile([P, HPC], F32R, tag="bk2")
        bv2_sb = cpool.tile([P, HPC], F32R, tag="bv2")
        id_sb = cpool.tile([P, P], F32R, tag="ident")

        nc.scalar.dma_start(out=wk_sb[:], in_=wk.ap())
        nc.scalar.dma_start(out=bk2_sb[:], in_=bk2.ap())
        nc.scalar.dma_start(out=bv2_sb[:], in_=bv2.ap())
        nc.scalar.dma_start(out=bqs_sb[:], in_=bqs.ap())
        nc.scalar.dma_start(out=id_sb[:], in_=ident.ap())
        nc.scalar.dma_start(out=wv_sb[:], in_=wv.ap())
        nc.scalar.dma_start(out=wq_sb[:], in_=wq.ap())
        nc.scalar.dma_start(out=wo_sb[:], in_=wo.ap())

        # ---- persistent activations ----
        qt_all = ppool.tile([P, HPC, S], F32R, tag="qt")   # Q^T per head
        kt_all = ppool.tile([P, HPC, S], F32R, tag="kt")   # K^T per head
        vt_all = ppool.tile([P, HPC, S], F32R, tag="vt")   # V^T per head
        v_all = ppool.tile([P, KT, DG], F32R, tag="v")     # V [k, dv]
        ctxT = ppool.tile([P, HPC, S], F32R, tag="ctxT")   # context^T

        # ---- projections: x^T streamed, 8 PSUM banks per phase ----
        with (
            tc.tile_pool(name="xs", bufs=3) as xpool,
            tc.tile_pool(name="pproj", bufs=8, space="PSUM") as ps_proj,
        ):
            def proj_phase(x_dram, w_sb, out_all, bias_sb, scale, phase):
                accs = [
                    ps_proj.tile([P, 512], mybir.dt.float32, tag="proj",
                                 name=f"acc_{phase}_{i}")
                    for i in range(HPC * QC)
                ]
                for dt_i in range(DT):
                    xt = xpool.tile([P, S], F32R, tag="xs", name=f"x_{phase}_{dt_i}")
                    nc.sync.dma_start(out=xt[:],
                                      in_=x_dram.ap()[dt_i * P:(dt_i + 1) * P, :])
                    for h in range(HPC):
                        for qc in range(QC):
                            nc.tensor.matmul(
                                accs[h * QC + qc][:],
                                w_sb[:, dt_i, h * HD:(h + 1) * HD],
                                xt[:, qc * 512:(qc + 1) * 512],
                                start=(dt_i == 0),
                                stop=(dt_i == DT - 1),
                            )
                for h in range(HPC):
                    for qc in range(QC):
                        nc.scalar.activation(
                            out_all[:, h, qc * 512:(qc + 1) * 512],
                            accs[h * QC + qc][:],
                            Identity,
                            bias=bias_sb[:, h:h + 1],
                            scale=scale,
                        )

            proj_phase(xkT, wk_sb, kt_all, bk2_sb, 1.0, "k")
            proj_phase(xvT, wv_sb, vt_all, bv2_sb, 1.0, "v")

            # transpose V^T -> V [k, dv] via PE (interleaved with Q-proj DMA)
            for h in range(HPC):
                for kt_i in range(KT):
                    tr = ps_proj.tile([P, P], F32R, tag="proj",
                                      name=f"tr_{h}_{kt_i}")
                    nc.tensor.transpose(
                        tr[:], vt_all[:, h, kt_i * P:(kt_i + 1) * P], id_sb[:]
                    )
                    nc.vector.tensor_copy(
                        v_all[:, kt_i, h * HD:(h + 1) * HD], tr[:]
                    )

            proj_phase(xqT, wq_sb, qt_all, bqs_sb, SCALE, "q")

        # ---- attention per head + output projection ----
        with (
            tc.tile_pool(name="attn_ps", bufs=1, space="PSUM") as ps_attn,
            tc.tile_pool(name="attn_sb", bufs=1) as apool,
        ):
            for h in range(HPC):
                ctx_ps = [
                    ps_attn.tile([P, 512], mybir.dt.float32, tag="ctx", bufs=4,
                                 name=f"ctxps_{h}_{qc}")
                    for qc in range(QC)
                ]
                for kt_i in range(KT):
                    for half in range(2):
                        at_ps = ps_attn.tile(
                            [P, 1024], mybir.dt.float32, tag="at", bufs=2,
                            name=f"atps_{h}_{kt_i}_{half}",
                        )
                        for sub in range(2):
                            qc = half * 2 + sub
                            nc.tensor.matmul(
                                at_ps[:, sub * 512:(sub + 1) * 512],
                                kt_all[:, h, kt_i * P:(kt_i + 1) * P],
                                qt_all[:, h, qc * 512:(qc + 1) * 512],
                                start=True,
                                stop=True,
                            )
                        at_sb = apool.tile([P, 1024], F32R, tag="at_sb", bufs=4,
                                           name=f"atsb_{h}_{kt_i}_{half}")
                        if half == 0:
                            nc.scalar.activation(at_sb[:], at_ps[:], Relu)
                        else:
                            nc.vector.tensor_scalar_max(at_sb[:], at_ps[:], 0.0)
                        nc.sync.dma_start(
                            out=attn_t.ap()[h, kt_i * P:(kt_i + 1) * P,
                                            half * 1024:(half + 1) * 1024],
                            in_=at_sb[:],
                        )
                        for sub in range(2):
                            qc = half * 2 + sub
                            nc.tensor.matmul(
                                ctx_ps[qc][:],
                                v_all[:, kt_i, h * HD:(h + 1) * HD],
                                at_sb[:, sub * 512:(sub + 1) * 512],
                                start=(kt_i == 0),
                                stop=(kt_i == KT - 1),
                            )
                for qc in range(QC):
                    nc.scalar.activation(
                        ctxT[:, h, qc * 512:(qc + 1) * 512], ctx_ps[qc][:], Copy
                    )

            # output projection
            for st in range(KT):
                o_ps = ps_attn.tile([P, D], mybir.dt.float32, tag="at", bufs=2,
                                    name=f"ops_{st}")
                for oc in range(2):
                    for h in range(HPC):
                        nc.tensor.matmul(
                            o_ps[:, oc * 512:(oc + 1) * 512],
                            ctxT[:, h, st * P:(st + 1) * P],
                            wo_sb[:, h, oc * 512:(oc + 1) * 512],
                            start=(h == 0),
                            stop=(h == HPC - 1),
                        )
                o_sb = apool.tile([P, D], F32R, tag="at_sb", bufs=4,
                                  name=f"osb_{st}")
                nc.vector.tensor_copy(o_sb[:], o_ps[:])
                nc.sync.dma_start(
                    out=out_partial.ap()[st * P:(st + 1) * P, :], in_=o_sb[:]
                )

    nc.compile()
    return nc


def _get_nc():
    global _CACHED_NC
    if _CACHED_NC is None:
        _CACHED_NC = _build_nc()
    return _CACHED_NC


def _warr(w):
    # [D, DG] -> [P, DT, DG] matching SBUF weight layout
    return np.ascontiguousarray(w.reshape(DT, P, -1).transpose(1, 0, 2))


def kernel(query, key, value, Wq, bq, Wk, bk, Wv, bv, Wo, bo):
    query = np.ascontiguousarray(np.asarray(query, dtype=np.float32))
    key = np.ascontiguousarray(np.asarray(key, dtype=np.float32))
    value = np.ascontiguousarray(np.asarray(value, dtype=np.float32))
    Wq = np.asarray(Wq, dtype=np.float32)
    bq = np.asarray(bq, dtype=np.float32)
    Wk = np.asarray(Wk, dtype=np.float32)
    bk = np.asarray(bk, dtype=np.float32)
    Wv = np.asarray(Wv, dtype=np.float32)
    bv = np.asarray(bv, dtype=np.float32)
    Wo = np.asarray(Wo, dtype=np.float32)
    bo = np.asarray(bo, dtype=np.float32)

    nc = _get_nc()

    xT = {}
    for name, x in (("q", query), ("k", key), ("v", value)):
        xT[name] = [np.ascontiguousarray(x[b].T) for b in range(B)]
    ident = np.eye(P, dtype=np.float32)

    in_maps = []
    for c in range(NCORES):
        b, hg = divmod(c, 4)
        sl = slice(hg * DG, (hg + 1) * DG)
        in_maps.append({
            "xqT": xT["q"][b],
            "xkT": xT["k"][b],
            "xvT": xT["v"][b],
            "wq": _warr(Wq[:, sl]),
            "wk": _warr(Wk[:, sl]),
            "wv": _warr(Wv[:, sl]),
            "wo": np.ascontiguousarray(
                Wo[sl, :].reshape(HPC, P, D).transpose(1, 0, 2)
            ),
            "bqs": np.ascontiguousarray((bq[sl] * SCALE).reshape(HPC, P).T),
            "bk2": np.ascontiguousarray(bk[sl].reshape(HPC, P).T),
            "bv2": np.ascontiguousarray(bv[sl].reshape(HPC, P).T),
            "ident": ident,
        })

    res = run_bass_kernel_spmd(nc, in_maps, core_ids=list(range(NCORES)))

    attn = np.empty((B, H, S, S), dtype=np.float32)
    output = np.zeros((B, S, D), dtype=np.float32)
    for c in range(NCORES):
        b, hg = divmod(c, 4)
        r = res.results[c]
        at = r["attn_t"]  # [HPC, S(key), S(query)]
        for h in range(HPC):
            attn[b, hg * HPC + h] = at[h].T
        output[b] += r["out_partial"]
    output += bo[None, None, :]
    return output, attn
